# revision 3
# baseline (speedup 1.0000x reference)
import numpy as np
import jax
import jax.numpy as jnp
from functools import partial

N, B, E, RES = 256, 8, 1024, 64
NC = 8
SH = N // NC  # 32 nodes per core


def _conv(x, w, b, s=1):
    y = jax.lax.conv_general_dilated(x, w, (s, s), [(1, 1), (1, 1)],
                                     dimension_numbers=("NCHW", "OIHW", "NCHW"))
    return y + b[None, :, None, None]


def _gn(x, g, be, eps=1e-5):
    mu = jnp.mean(x, axis=(1, 2, 3), keepdims=True)
    var = jnp.mean((x - mu) ** 2, axis=(1, 2, 3), keepdims=True)
    xn = (x - mu) * jax.lax.rsqrt(var + eps)
    return xn * g[None, :, None, None] + be[None, :, None, None]


def _blk(x, p, s=1):
    x = _conv(x, p["w"], p["b"], s)
    if "g" in p:
        x = _gn(x, p["g"], p["be"])
    return jax.nn.leaky_relu(x, 0.1)


def _seq(x, blocks, strides):
    for p, s in zip(blocks, strides):
        x = _blk(x, p, s)
    return x


def _shard_body(x_sh, y_sh, img64, img32, img16, img8, Asrc, Ssh, params):
    """Runs on each core. x_sh: [SH,1,R,R] local nodes; y_sh: [SH,32];
    Asrc: [N, SH] pooling partial matrix (dst all, src local) already /n_pos;
    Ssh: [B, SH] sample pooling matrix for local nodes."""
    V = SH
    y = jnp.broadcast_to(y_sh[:, :, None, None], (V, 32, RES, RES))
    h = jnp.concatenate([x_sh, y], axis=1)
    h = _seq(h, params["encoder"], [1, 1, 1, 1])

    def cmp_blk(feats, extra, blocks):
        # pooled[d] = sum_s A[d,s] feats[s]; distributed: psum over cores of
        # Asrc_local @ feats_local, then slice own rows via ppermute-free trick:
        part = jnp.einsum('ds,schw->dchw', Asrc, feats)          # [N,C,H,W] local partial
        pooled = jax.lax.psum(part, axis_name='c')               # replicated full
        pooled = jax.lax.dynamic_slice_in_dim(
            pooled, jax.lax.axis_index('c') * SH, SH, axis=0)    # own rows
        ex = jnp.broadcast_to(extra[None], (V,) + extra.shape)
        return _seq(jnp.concatenate([feats, pooled, ex], axis=1), blocks, [1, 1, 1])

    h = cmp_blk(h, img64, params["cmp"][0]); h = _blk(h, params["cmp_down"][0], 2)
    h = cmp_blk(h, img32, params["cmp"][1]); h = _blk(h, params["cmp_down"][1], 2)
    h = cmp_blk(h, img16, params["cmp"][2]); h = _blk(h, params["cmp_down"][2], 2)
    h = jnp.concatenate([h, jnp.broadcast_to(img8[None], (V,) + img8.shape)], axis=1)

    # global head: per-sample sum over nodes (distributed)
    xg = jnp.einsum('sn,nchw->schw', Ssh, h)
    xg = jax.lax.psum(xg, axis_name='c')
    xg = _seq(xg, params["global_dec"], [2, 2, 2]).reshape(-1, 128)
    vg = xg @ params["fcg_w"].T + params["fcg_b"]
    # local head
    xl = _seq(h, params["local_dec"], [2, 2, 2]).reshape(V, 128)
    xl = jnp.einsum('sn,nd->sd', Ssh, xl)
    xl = jax.lax.psum(xl, axis_name='c')
    vl = xl @ params["fcl_w"].T + params["fcl_b"]
    return vg + vl  # [B,1] replicated


def _np_conv(x, w, b, s=1):
    # x: [V,C,H,W] numpy 3x3 conv pad 1, stride s
    V, C, H, W = x.shape
    O = w.shape[0]
    xp = np.zeros((V, C, H + 2, W + 2), np.float32)
    xp[:, :, 1:H + 1, 1:W + 1] = x
    Ho, Wo = (H + s - 1) // s, (W + s - 1) // s
    y = np.zeros((V, O, Ho, Wo), np.float32)
    for p in range(3):
        for q in range(3):
            xs = xp[:, :, p:p + H:s, q:q + W:s]
            y += np.einsum('oc,vchw->vohw', w[:, :, p, q], xs, optimize=True)
    return y + b[None, :, None, None]


def _np_seq(x, blocks, strides):
    for p, s in zip(blocks, strides):
        x = _np_conv(x, p["w"], p["b"], s)
        if "g" in p:
            mu = x.mean(axis=(1, 2, 3), keepdims=True)
            var = ((x - mu) ** 2).mean(axis=(1, 2, 3), keepdims=True)
            x = (x - mu) / np.sqrt(var + 1e-5)
            x = x * p["g"][None, :, None, None] + p["be"][None, :, None, None]
        x = np.where(x > 0, x, 0.1 * x).astype(np.float32)
    return x


_CACHE = {}


def _get_fn():
    if 'fn' not in _CACHE:
        _CACHE['fn'] = jax.pmap(_shard_body, axis_name='c',
                                in_axes=(0, 0, None, None, None, None, 0, 0, None),
                                out_axes=None)
    return _CACHE['fn']


def kernel(x, given_y, topo_vecs, given_b, params, given_w, nd_to_sample):
    x = np.asarray(x, np.float32)
    params = jax.tree.map(lambda a: jnp.asarray(a, jnp.float32), params)

    # host prep: conditioning projection (tiny), pooling matrices, pyramid input
    yfull = np.concatenate([np.asarray(given_y, np.float32),
                            np.asarray(topo_vecs, np.float32)], axis=1)
    y = yfull @ np.asarray(params["l1_w"], np.float32).T + np.asarray(params["l1_b"], np.float32)

    edges = np.asarray(given_w).reshape(-1, 3)
    src = np.concatenate([edges[:, 0], edges[:, 2]]).astype(np.int64)
    dst = np.concatenate([edges[:, 2], edges[:, 0]]).astype(np.int64)
    A = np.zeros((N, N), np.float32)
    np.add.at(A, (dst, src), 1.0)
    A /= float(edges.shape[0])
    S = np.zeros((B, N), np.float32)
    S[np.asarray(nd_to_sample).astype(np.int64), np.arange(N)] = 1.0

    # boundary pyramid computed once (shared across nodes) — tiny, on host
    gb = np.asarray(given_b, np.float32)[None, None]
    np_params = jax.tree.map(lambda a: np.asarray(a, np.float32), params)
    img64 = _np_seq(gb, np_params["enc_same"], [1, 1])
    img32 = _np_seq(img64, np_params["enc_down"][0], [2, 1])
    img16 = _np_seq(img32, np_params["enc_down"][1], [2, 1])
    img8 = _np_seq(img16, np_params["enc_down"][2], [2, 1])

    x_sh = x.reshape(NC, SH, 1, RES, RES)
    y_sh = y.reshape(NC, SH, 32).astype(np.float32)
    A_sh = A.reshape(N, NC, SH).transpose(1, 0, 2).copy()   # [NC, N, SH]
    S_sh = S.reshape(B, NC, SH).transpose(1, 0, 2).copy()   # [NC, B, SH]

    fn = _get_fn()
    out = fn(jnp.asarray(x_sh), jnp.asarray(y_sh), img64[0], img32[0],
             img16[0], img8[0], jnp.asarray(A_sh), jnp.asarray(S_sh), params)
    return np.asarray(out, np.float32)
